# revision 3
# baseline (speedup 1.0000x reference)
"""Two-layer GAT on 8 Trainium2 NeuronCores via Bass/Tile.

Baseline structure (per-slot-column [128,1] indirect DMA gathers, degree-sorted
relabeling, ~1.3% slot padding) with three upgrades validated on HW:
 - bf16 table rows of 80 elems (160B) instead of f32 512B: 3.2x less AllGather
   and gather DMA traffic (descriptors fetch 72 of 80 elems).
 - weight augmentation: es/ed attention scores come out of the PE matmul
   (extra columns W@a_src, W@a_dst) - no DVE reductions in the node phase.
 - edge-phase softmax/weighted-sum in bf16 (2x DVE throughput), exp on ACT.
"""
import sys

sys.path.insert(0, "/opt/trn_rl_repo")

import numpy as np

import concourse.bass as bass
import concourse.bacc as bacc
import concourse.tile as tile
from concourse import mybir
from concourse.bass import AP, IndirectOffsetOnAxis
from concourse.masks import make_identity

F32 = mybir.dt.float32
BF16 = mybir.dt.bfloat16
I32 = mybir.dt.int32
AX = mybir.AxisListType.X
OP = mybir.AluOpType
AF = mybir.ActivationFunctionType

N = 100_000
F_IN = 512
H1, FH1 = 8, 8
D1 = H1 * FH1          # 64
C = 64
NCORES = 8
NLR = N // NCORES      # 12500
PW = 128
NWIN = (NLR + PW - 1) // PW   # 98
NL = NWIN * PW         # 12544
GT = NCORES * NL       # 100352
RL = 72                # table row stride in bf16 elems (144 B): h(64)|es(8)
RF = 72                # fetched elems per gather descriptor
NEG = -1.0e30
XB = 2
SB = 7
HW_ROWS = 49 * PW      # AllGather chunk A: first 49 windows' rows (6272/core)


# ---------------------------------------------------------------- host planning
def _plan(edge_index):
    src = np.concatenate([edge_index[0], np.arange(N)]).astype(np.int64)
    dst = np.concatenate([edge_index[1], np.arange(N)]).astype(np.int64)
    owner = dst // NLR

    orders, posmaps, per_core = [], [], []
    for c in range(NCORES):
        m = owner == c
        s_c, d_c = src[m], dst[m] - c * NLR
        per_core.append((s_c, d_c))
        deg = np.bincount(d_c, minlength=NLR)
        order = np.argsort(-deg, kind="stable")
        posmap = np.empty(NLR, dtype=np.int64)
        posmap[order] = np.arange(NLR)
        orders.append(order)
        posmaps.append(posmap)

    K = np.ones(NWIN, dtype=np.int64)
    for c in range(NCORES):
        deg = np.bincount(per_core[c][1], minlength=NLR)
        dpad = np.zeros(NL, dtype=np.int64)
        dpad[:NLR] = deg[orders[c]]
        K = np.maximum(K, dpad.reshape(NWIN, PW).max(axis=1))
    offs = np.concatenate([[0], np.cumsum(K)]).astype(np.int64)
    tot = int(offs[-1])

    idx_streams = []
    for c in range(NCORES):
        s_c, d_c = per_core[c]
        pos = posmaps[c][d_c]
        srow = np.empty(len(s_c), dtype=np.int64)
        so = s_c // NLR
        for o in range(NCORES):
            mo = so == o
            srow[mo] = o * NL + posmaps[o][s_c[mo] - o * NLR]
        # table rows are laid out in two AllGather chunks:
        #   chunk A rows [0, 8H): core o's positions [0, H) at o*H + pos
        #   chunk B rows [8H, GT): core o's positions [H, NL) at
        #                          8H + o*(NL-H) + (pos-H)
        H = HW_ROWS
        srow_pos = srow % NL
        srow_core = srow // NL
        srow = np.where(
            srow_pos < H,
            srow_core * H + srow_pos,
            NCORES * H + srow_core * (NL - H) + (srow_pos - H),
        )
        ordd = np.argsort(pos, kind="stable")
        pos_s, srow_s = pos[ordd], srow[ordd]
        cnt = np.bincount(pos_s, minlength=NL)
        starts = np.concatenate([[0], np.cumsum(cnt)])[:-1]
        k_of = np.arange(len(pos_s)) - starts[pos_s]
        pad_row = NCORES * H + c * (NL - H) + (NL - 1 - H)
        idxmat = np.full((128, tot), pad_row, dtype=np.int32)
        idxmat[pos_s & 127, offs[pos_s >> 7] + k_of] = srow_s
        idx_streams.append(idxmat)

    return {"orders": orders, "K": K, "offs": offs, "tot": tot, "idx": idx_streams}


def _apx(base: AP, off: int, dims) -> AP:
    return AP(base.tensor, base.offset + off, [list(base.ap[0])] + [list(d) for d in dims])


# ---------------------------------------------------------------- device build
def _build(K, tot, offs):
    K = [int(v) for v in K]
    offs = [int(v) for v in offs]
    K0 = max(K)

    nc = bacc.Bacc("TRN2", target_bir_lowering=False, debug=False, num_devices=NCORES)

    xT = nc.dram_tensor("xT", [F_IN, NL], F32, kind="ExternalInput")
    w1 = nc.dram_tensor("w1", [F_IN, 80], F32, kind="ExternalInput")
    w2 = nc.dram_tensor("w2", [D1, 66], F32, kind="ExternalInput")
    cvecd = nc.dram_tensor("cvecd", [128, 128], F32, kind="ExternalInput")
    idxd = nc.dram_tensor("idxd", [128, tot], I32, kind="ExternalInput")
    outd = nc.dram_tensor("outv", [NL, C], F32, kind="ExternalOutput")

    t1b = nc.dram_tensor("t1b", [NL, RL], BF16)
    T1 = nc.dram_tensor("T1", [GT, RL], BF16, addr_space="Shared")
    t2b = nc.dram_tensor("t2b", [NL, RL], BF16)
    T2 = nc.dram_tensor("T2", [GT, RL], BF16, addr_space="Shared")

    with tile.TileContext(nc) as tc:
        with (
            tc.tile_pool(name="consts", bufs=1) as cpool,
            tc.tile_pool(name="persist", bufs=1) as ppool,
            tc.tile_pool(name="xload", bufs=2) as xpool,
            tc.tile_pool(name="stg", bufs=2) as stgpool,
            tc.tile_pool(name="gpool", bufs=3) as gpool,
            tc.tile_pool(name="zpool", bufs=2) as zpool,
            tc.tile_pool(name="small", bufs=2) as spool,
            tc.tile_pool(name="psum", bufs=2, space="PSUM") as pspool,
        ):
            # ---- constants
            w1sb = cpool.tile([128, 4 * 80], F32)
            nc.sync.dma_start(
                out=w1sb[:].rearrange("p (cc d) -> p cc d", cc=4),
                in_=w1[:, :].rearrange("(cc p) d -> p cc d", p=128),
            )
            w2sb = cpool.tile([128, 66], F32)
            nc.sync.dma_start(out=w2sb[:D1, :], in_=w2[:, :])
            cv = cpool.tile([128, 128], F32)
            nc.sync.dma_start(out=cv[:], in_=cvecd[:, :])
            b1s = cv[:, 0:64]
            b2s = cv[:, 64:128]
            ident = cpool.tile([128, 128], F32)
            make_identity(nc, ident[:])
            phant = cpool.tile([128, RL], BF16)
            nc.vector.memset(phant[:], NEG)
            idxs = cpool.tile([128, tot], I32)
            nc.sync.dma_start(out=idxs[:], in_=idxd[:, :])

            # ---- persistent
            x2st = ppool.tile([128, NWIN * D1], F32)
            edt1 = ppool.tile([128, NWIN * H1], BF16)
            edt2 = ppool.tile([128, NWIN], BF16)

            def node_phase(layer):
                tb, Tg = (t1b, T1) if layer == 1 else (t2b, T2)
                for sb in range(0, NWIN, SB):
                    stg = stgpool.tile([128, SB * RL], BF16, tag="stg")
                    nc.vector.memset(stg[:], 0.0)
                    for w in range(sb, sb + SB):
                        wl = w - sb
                        if layer == 1:
                            if w % XB == 0:
                                xb = xpool.tile([128, 4 * XB * 128], F32, tag="xb")
                                nc.sync.dma_start(
                                    out=xb[:].rearrange("p (cc n) -> p cc n", cc=4),
                                    in_=xT[:, w * 128 : (w + XB) * 128].rearrange(
                                        "(cc p) n -> p cc n", p=128
                                    ),
                                )
                            ph = pspool.tile([128, 80], F32, tag="ph")
                            nn = XB * 128
                            for cc in range(4):
                                nc.tensor.matmul(
                                    out=ph[:],
                                    lhsT=_apx(xb[:], cc * nn + (w % XB) * 128, [[1, 128]]),
                                    rhs=_apx(w1sb[:], cc * 80, [[1, 80]]),
                                    start=(cc == 0),
                                    stop=(cc == 3),
                                )
                            nc.vector.tensor_copy(
                                out=_apx(stg[:], wl * RL, [[1, 72]]),
                                in_=ph[:, 0:72],
                            )
                            nc.vector.tensor_copy(
                                out=_apx(edt1[:], w * H1, [[1, H1]]),
                                in_=ph[:, 72:80],
                            )
                        else:
                            pt = pspool.tile([64, 128], F32, tag="pt")
                            nc.tensor.transpose(
                                out=pt[:],
                                in_=_apx(x2st[:], w * D1, [[1, D1]]),
                                identity=ident[:],
                            )
                            x1t = spool.tile([64, 128], F32, tag="x1t")
                            nc.vector.tensor_copy(out=x1t[:], in_=pt[:])
                            ph = pspool.tile([128, 66], F32, tag="ph2")
                            nc.tensor.matmul(
                                out=ph[:], lhsT=x1t[:], rhs=w2sb[:D1, :],
                                start=True, stop=True,
                            )
                            nc.vector.tensor_copy(
                                out=_apx(stg[:], wl * RL, [[1, 65]]),
                                in_=ph[:, 0:65],
                            )
                            nc.vector.tensor_copy(
                                out=_apx(edt2[:], w, [[1, 1]]),
                                in_=ph[:, 65:66],
                            )
                    nc.sync.dma_start(
                        out=tb[sb * 128 : (sb + SB) * 128, :].rearrange(
                            "(w p) r -> p w r", p=128
                        ),
                        in_=stg[:].rearrange("p (w r) -> p w r", w=SB),
                    )
                    if (sb + SB) * 128 == HW_ROWS:
                        # first half staged: AllGather chunk A overlaps the
                        # rest of the node phase
                        nc.gpsimd.collective_compute(
                            "AllGather", OP.bypass,
                            replica_groups=[list(range(NCORES))],
                            ins=[tb[:HW_ROWS, :]],
                            outs=[Tg[: NCORES * HW_ROWS, :]],
                        )
                nc.sync.dma_start(out=tb[NLR:NL, :], in_=phant[: NL - NLR, :])
                nc.gpsimd.collective_compute(
                    "AllGather", OP.bypass,
                    replica_groups=[list(range(NCORES))],
                    ins=[tb[HW_ROWS:NL, :]],
                    outs=[Tg[NCORES * HW_ROWS :, :]],
                )

            def edge_phase(layer):
                Tg = T1 if layer == 1 else T2
                for w in range(NWIN):
                    Kw = K[w]
                    G = gpool.tile([128, K0 * RL], BF16, tag="G")
                    for k in range(Kw):
                        nc.gpsimd.indirect_dma_start(
                            out=_apx(G[:], k * RL, [[1, RF]]),
                            out_offset=None,
                            in_=Tg[:, :],
                            in_offset=IndirectOffsetOnAxis(
                                ap=idxs[:, offs[w] + k : offs[w] + k + 1], axis=0
                            ),
                        )
                    dn = spool.tile([128, 16], F32, tag="dn")
                    ot = spool.tile([128, 64], F32, tag="ot")
                    if layer == 1:
                        z = zpool.tile([128, K0 * H1], BF16, tag="z")
                        nc.vector.tensor_tensor(
                            out=_apx(z[:], 0, [[H1, Kw], [1, H1]]),
                            in0=_apx(G[:], 64, [[RL, Kw], [1, H1]]),
                            in1=_apx(edt1[:], w * H1, [[0, Kw], [1, H1]]),
                            op=OP.add)
                        zf = _apx(z[:], 0, [[1, H1 * Kw]])
                        nc.vector.scalar_tensor_tensor(
                            out=zf, in0=zf, scalar=0.2, in1=zf, op0=OP.mult, op1=OP.max)
                        nc.scalar.activation(out=zf, in_=zf, func=AF.Exp)
                        nc.vector.tensor_reduce(
                            out=dn[:, 0:H1],
                            in_=_apx(z[:], 0, [[1, H1], [H1, Kw]]),
                            axis=AX, op=OP.add)
                        nc.vector.tensor_scalar_add(dn[:, 0:H1], dn[:, 0:H1], 1e-30)
                        nc.vector.reciprocal(out=dn[:, 8:16], in_=dn[:, 0:H1])
                        gh = _apx(G[:], 0, [[RL, Kw], [FH1, H1], [1, FH1]])
                        nc.vector.tensor_tensor(
                            out=gh, in0=gh,
                            in1=_apx(z[:], 0, [[H1, Kw], [1, H1], [0, FH1]]),
                            op=OP.mult)
                        nc.vector.tensor_reduce(
                            out=ot[:],
                            in_=_apx(G[:], 0, [[FH1, H1], [1, FH1], [RL, Kw]]),
                            axis=AX, op=OP.add)
                        nc.vector.tensor_tensor(
                            out=_apx(x2st[:], w * D1, [[1, D1]]),
                            in0=ot[:],
                            in1=_apx(dn[:], 8, [[1, H1], [0, FH1]]),
                            op=OP.mult)
                    else:
                        z = zpool.tile([128, K0 * H1], BF16, tag="z")
                        nc.vector.tensor_tensor(
                            out=_apx(z[:], 0, [[1, Kw]]),
                            in0=_apx(G[:], 64, [[RL, Kw]]),
                            in1=_apx(edt2[:], w, [[0, Kw]]),
                            op=OP.add)
                        zf = _apx(z[:], 0, [[1, Kw]])
                        nc.vector.scalar_tensor_tensor(
                            out=zf, in0=zf, scalar=0.2, in1=zf, op0=OP.mult, op1=OP.max)
                        nc.scalar.activation(out=zf, in_=zf, func=AF.Exp)
                        nc.vector.tensor_reduce(out=dn[:, 0:1], in_=zf, axis=AX, op=OP.add)
                        nc.vector.tensor_scalar_add(dn[:, 0:1], dn[:, 0:1], 1e-30)
                        nc.vector.reciprocal(out=dn[:, 1:2], in_=dn[:, 0:1])
                        gh = _apx(G[:], 0, [[RL, Kw], [1, C]])
                        nc.vector.tensor_tensor(
                            out=gh, in0=gh,
                            in1=_apx(z[:], 0, [[1, Kw], [0, C]]), op=OP.mult)
                        nc.vector.tensor_reduce(
                            out=ot[:],
                            in_=_apx(G[:], 0, [[1, C], [RL, Kw]]),
                            axis=AX, op=OP.add)
                        nc.vector.tensor_tensor(
                            out=_apx(x2st[:], w * C, [[1, C]]),
                            in0=ot[:],
                            in1=_apx(dn[:], 1, [[0, C]]),
                            op=OP.mult)

            # ================= layer 1 =================
            node_phase(1)
            edge_phase(1)
            for g in range(0, NWIN, SB):
                xs = _apx(x2st[:], g * D1, [[1, SB * D1]])
                nc.vector.tensor_tensor(
                    out=xs, in0=xs, in1=_apx(b1s, 0, [[0, SB], [1, D1]]), op=OP.add)
                tmp = spool.tile([128, SB * D1], F32, tag="tail")
                tf = _apx(tmp[:], 0, [[1, SB * D1]])
                nc.vector.tensor_scalar_min(tf, xs, 0.0)
                nc.scalar.activation(out=tf, in_=tf, func=AF.Exp)
                nc.vector.tensor_scalar_max(xs, xs, 0.0)
                nc.vector.scalar_tensor_tensor(
                    out=xs, in0=tf, scalar=-1.0, in1=xs, op0=OP.add, op1=OP.add)

            # ================= layer 2 =================
            node_phase(2)
            edge_phase(2)
            for g in range(0, NWIN, SB):
                xs = _apx(x2st[:], g * C, [[1, SB * C]])
                nc.vector.tensor_tensor(
                    out=xs, in0=xs, in1=_apx(b2s, 0, [[0, SB], [1, C]]), op=OP.add)
                rmx = spool.tile([128, SB], F32, tag="rmx")
                nc.vector.tensor_reduce(
                    out=rmx[:], in_=_apx(x2st[:], g * C, [[C, SB], [1, C]]),
                    axis=AX, op=OP.max)
                nc.vector.tensor_tensor(
                    out=xs, in0=xs, in1=_apx(rmx[:], 0, [[1, SB], [0, C]]),
                    op=OP.subtract)
                tmp = spool.tile([128, SB * C], F32, tag="tail")
                tf = _apx(tmp[:], 0, [[1, SB * C]])
                nc.scalar.activation(out=tf, in_=xs, func=AF.Exp)
                nc.vector.tensor_reduce(
                    out=rmx[:], in_=_apx(tmp[:], 0, [[C, SB], [1, C]]),
                    axis=AX, op=OP.add)
                nc.scalar.activation(out=rmx[:], in_=rmx[:], func=AF.Ln)
                nc.vector.tensor_tensor(
                    out=xs, in0=xs, in1=_apx(rmx[:], 0, [[1, SB], [0, C]]),
                    op=OP.subtract)
            nc.sync.dma_start(
                out=outd[:, :].rearrange("(w p) f -> p w f", p=128),
                in_=x2st[:].rearrange("p (w f) -> p w f", w=NWIN),
            )

    nc.compile()
    return nc


# ---------------------------------------------------------------- PJRT runner
def _make_runner(nc):
    import jax
    from jax.sharding import Mesh, PartitionSpec, NamedSharding
    from jax.experimental.shard_map import shard_map
    from concourse import bass2jax
    from concourse.bass2jax import _bass_exec_p, install_neuronx_cc_hook

    install_neuronx_cc_hook()
    partition_name = nc.partition_id_tensor.name if nc.partition_id_tensor else None
    in_names, out_names, out_avals = [], [], []
    for alloc in nc.m.functions[0].allocations:
        if not isinstance(alloc, mybir.MemoryLocationSet):
            continue
        name = alloc.memorylocations[0].name
        if alloc.kind == "ExternalInput":
            if name != partition_name:
                in_names.append(name)
        elif alloc.kind == "ExternalOutput":
            out_avals.append(
                jax.core.ShapedArray(tuple(alloc.tensor_shape), mybir.dt.np(alloc.dtype))
            )
            out_names.append(name)
    n_params = len(in_names)
    all_in = list(in_names) + list(out_names)
    if partition_name is not None:
        all_in.append(partition_name)

    def _body(*args):
        operands = list(args)
        if partition_name is not None:
            operands.append(bass2jax.partition_id_tensor())
        return tuple(
            _bass_exec_p.bind(
                *operands,
                out_avals=tuple(out_avals),
                in_names=tuple(all_in),
                out_names=tuple(out_names),
                lowering_input_output_aliases=(),
                sim_require_finite=True,
                sim_require_nnan=True,
                nc=nc,
            )
        )

    devices = jax.devices()[:NCORES]
    mesh = Mesh(np.asarray(devices), ("core",))
    n_outs = len(out_names)
    sharded = jax.jit(
        shard_map(
            _body, mesh=mesh,
            in_specs=(PartitionSpec("core"),) * (n_params + n_outs),
            out_specs=(PartitionSpec("core"),) * n_outs,
            check_rep=False,
        ),
        keep_unused=True,
    )
    sharding = NamedSharding(mesh, PartitionSpec("core"))

    def run(in_maps):
        import jax as _jax

        per_core = [[np.asarray(m[nm]) for nm in in_names] for m in in_maps]
        concat_in = [
            np.concatenate([per_core[c][i] for c in range(NCORES)], axis=0)
            for i in range(n_params)
        ]
        concat_zero = [
            np.zeros((NCORES * a.shape[0], *a.shape[1:]), a.dtype) for a in out_avals
        ]
        args = [_jax.device_put(x, sharding) for x in concat_in + concat_zero]
        out = sharded(*args)
        _jax.block_until_ready(out)
        return (
            [
                {
                    nm: np.asarray(out[i]).reshape(NCORES, *out_avals[i].shape)[c]
                    for i, nm in enumerate(out_names)
                }
                for c in range(NCORES)
            ],
            sharded,
            args,
        )

    return run


_CACHE = {}


def _get_compiled(K, tot, offs):
    key = (tot, tuple(int(v) for v in K))
    if key not in _CACHE:
        nc = _build(K, tot, offs)
        _CACHE[key] = (nc, _make_runner(nc))
    return _CACHE[key]


def _prep_inputs(x, plan, W1, att1_src, att1_dst, b1, W2, att2_src, att2_dst, b2):
    W1 = np.asarray(W1, np.float32)
    W2 = np.asarray(W2, np.float32)
    a1s = np.asarray(att1_src, np.float32)
    a1d = np.asarray(att1_dst, np.float32)
    a2s = np.asarray(att2_src, np.float32).reshape(C)
    a2d = np.asarray(att2_dst, np.float32).reshape(C)
    W1r = W1.reshape(F_IN, H1, FH1)
    w1aug = np.concatenate(
        [W1, np.einsum("khf,hf->kh", W1r, a1s), np.einsum("khf,hf->kh", W1r, a1d)],
        axis=1,
    ).astype(np.float32)
    w2aug = np.concatenate(
        [W2, (W2 @ a2s)[:, None], (W2 @ a2d)[:, None]], axis=1
    ).astype(np.float32)
    cvec = np.zeros((128, 128), np.float32)
    cvec[:, 0:64] = np.asarray(b1, np.float32).reshape(1, D1)
    cvec[:, 64:128] = np.asarray(b2, np.float32).reshape(1, C)
    in_maps = []
    for c in range(NCORES):
        order = plan["orders"][c]
        xp = np.zeros((NL, F_IN), np.float32)
        xp[:NLR] = x[c * NLR : (c + 1) * NLR][order]
        in_maps.append(
            {
                "xT": np.ascontiguousarray(xp.T),
                "w1": w1aug,
                "w2": w2aug,
                "cvecd": cvec,
                "idxd": plan["idx"][c],
            }
        )
    return in_maps


def kernel(x, edge_index, W1, att1_src, att1_dst, b1, W2, att2_src, att2_dst, b2):
    x = np.asarray(x, np.float32)
    edge_index = np.asarray(edge_index)
    plan = _plan(edge_index)
    nc, run = _get_compiled(plan["K"], plan["tot"], plan["offs"])
    in_maps = _prep_inputs(
        x, plan,
        np.asarray(W1), np.asarray(att1_src), np.asarray(att1_dst), np.asarray(b1),
        np.asarray(W2), np.asarray(att2_src), np.asarray(att2_dst), np.asarray(b2),
    )
    results, _, _ = run(in_maps)
    out = np.empty((N, C), np.float32)
    for c in range(NCORES):
        out[c * NLR + plan["orders"][c]] = results[c]["outv"][:NLR]
    return out
